# revision 5
# baseline (speedup 1.0000x reference)
"""Trainium2 Bass kernel for nn_MultiHeadAttention_47227460387052.

Multi-head attention (B=2, S=2048, D=1024, H=16, DK=64) with a non-standard
softmax scale (*S_k) and an additive bool mask.  Returns BOTH the projected
output [B,S,D] and the attention weights [B,H,S,S] (512 MB), so the kernel is
dominated by the attn-weight write stream.

Sharding: 8 cores = (batch, head-group): core c handles batch c//4 and heads
4*(c%4) .. 4*(c%4)+4.  Each core:
  - projects q/k (transposed layout, f32) and v (natural layout, bf16)
  - per 128-row q-tile: scores on PE (float32r), mask-add on DVE (PSUM src),
    exp on ACT with accumulated row sums, normalization (x * 2048/Z) on
    DVE/ACT, f32 attn written to HBM, a bf16 copy block-transposed via the
    DMA xbar for the ctx matmul
  - ctx^T = sum_k v_chunk^T @ attnT_chunk on PE (bf16)
  - out_p^T = Wo_slice @ ctx^T on PE (f32r), written transposed; the host
    transposes and sums the 4 partials per batch.
v/o biases never touch the device: attn rows sum to exactly 2048, so the
v-bias contributes the constant 2048*(bv @ Wo.T), folded in on the host.
"""

import os

import numpy as np
import ml_dtypes

B, S, D, H = 2, 2048, 1024, 16
DK = D // H  # 64
NCORES = 8
GROUPS_PER_B = NCORES // B  # 4 cores per batch
HPC = H // GROUPS_PER_B  # 4 heads per core
NT = S // 128  # 16 q tiles
NC = S // 128  # 16 k chunks
DCH = D // 128  # 8 contraction chunks for projections
MASK_NEG = np.float32(-1e9)

_CACHE = {}


def _build_module():
    """Build the Bass module (one program, SPMD across 8 cores)."""
    import concourse.bass as bass
    import concourse.mybir as mybir
    import concourse.tile as tile
    from concourse import bacc

    f32 = mybir.dt.float32
    f32r = mybir.dt.float32r
    bf16 = mybir.dt.bfloat16
    AF = mybir.ActivationFunctionType
    OP = mybir.AluOpType

    nc = bacc.Bacc("TRN2", target_bir_lowering=False, debug=False,
                   num_devices=NCORES)

    # ---- DRAM I/O (per core) ----
    qt_d = nc.dram_tensor("qt", [D, S], f32r, kind="ExternalInput").ap()
    kt_d = nc.dram_tensor("kt", [D, S], f32r, kind="ExternalInput").ap()
    vt_d = nc.dram_tensor("vt", [D, S], bf16, kind="ExternalInput").ap()
    mstr_d = nc.dram_tensor("mstr", [S, S], bf16, kind="ExternalInput").ap()
    wqt_d = nc.dram_tensor("wqt", [D, HPC * DK], f32r, kind="ExternalInput").ap()
    wkt_d = nc.dram_tensor("wkt", [D, HPC * DK], f32r, kind="ExternalInput").ap()
    wvt_d = nc.dram_tensor("wvt", [D, HPC * DK], bf16, kind="ExternalInput").ap()
    wot_d = nc.dram_tensor("wot", [HPC * DK, D], f32r, kind="ExternalInput").ap()
    bqs_d = nc.dram_tensor("bqs", [HPC * DK, 1], f32, kind="ExternalInput").ap()
    bks_d = nc.dram_tensor("bks", [HPC * DK, 1], f32, kind="ExternalInput").ap()
    attnw_d = nc.dram_tensor("attnw", [HPC, S, S], f32, kind="ExternalOutput").ap()
    outpt_d = nc.dram_tensor("outpt", [D, S], f32, kind="ExternalOutput").ap()

    with tile.TileContext(nc) as tc:
        with (
            tc.tile_pool(name="psum", bufs=4, space="PSUM") as pp,
            tc.tile_pool(name="sb", bufs=2) as sb,
        ):
            # ---------- weights to SBUF ----------
            wqt_sb = sb.tile([128, DCH, 256], f32r, name="wqt_sb",
                             tag="xstream", bufs=4)
            wkt_sb = sb.tile([128, DCH, 256], f32r, name="wkt_sb",
                             tag="xstream", bufs=4)
            wvt_sb = sb.tile([128, DCH, 256], bf16, name="wvt_sb",
                             tag="wvt", bufs=1)
            wot_sb = sb.tile([128, 2, D], f32r, name="wot_sb", tag="wot", bufs=1)
            bq_sb = sb.tile([128, 2], f32, name="bq_sb", tag="bias", bufs=2)
            bk_sb = sb.tile([128, 2], f32, name="bk_sb", tag="bias", bufs=2)
            nc.sync.dma_start(wqt_sb[:], wqt_d.rearrange("(c p) m -> p c m", p=128))
            nc.sync.dma_start(wkt_sb[:], wkt_d.rearrange("(c p) m -> p c m", p=128))
            nc.sync.dma_start(wvt_sb[:], wvt_d.rearrange("(c p) m -> p c m", p=128))
            nc.sync.dma_start(wot_sb[:], wot_d.rearrange("(c p) m -> p c m", p=128))
            nc.sync.dma_start(bq_sb[:], bqs_d.rearrange("(c p) m -> p (c m)", p=128))
            nc.sync.dma_start(bk_sb[:], bks_d.rearrange("(c p) m -> p (c m)", p=128))

            # ---------- projections ----------
            # qT/kT: [128(2 heads x 64), S] f32 per pair; vT -> xbar -> v natural
            qT = [sb.tile([128, S], f32r, name=f"qT{p}", tag="qkT", bufs=4)
                  for p in range(2)]
            kT = [sb.tile([128, S], f32r, name=f"kT{p}", tag="qkT", bufs=4)
                  for p in range(2)]
            vTp = [sb.tile([128, S], bf16, name=f"vTp{p}", tag="bf4k", bufs=6)
                   for p in range(2)]
            v_sb = [sb.tile([128, NC, 128], bf16, name=f"v_sb{p}", tag="bf4k",
                            bufs=6)
                    for p in range(2)]

            def project(x_d, w_sb, outs, bias_sb, w_dt):
                acc = [[pp.tile([128, 1024], f32, name=f"acc{p}{s}", tag="ps",
                                bufs=4)
                        for s in range(2)] for p in range(2)]
                for cd in range(DCH):
                    xch = sb.tile([128, S], w_dt, name=f"xch{cd}",
                                  tag="xstream", bufs=4)
                    nc.sync.dma_start(xch[:], x_d[cd * 128:(cd + 1) * 128, :])
                    for pair in range(2):
                        lhsT = w_sb[:, cd, pair * 128:(pair + 1) * 128]
                        for sh in range(2):
                            for n in range(2):
                                rhs = xch[:, sh * 1024 + n * 512:
                                          sh * 1024 + (n + 1) * 512]
                                nc.tensor.matmul(
                                    acc[pair][sh][:, n * 512:(n + 1) * 512],
                                    lhsT, rhs,
                                    start=(cd == 0), stop=(cd == DCH - 1))
                for pair in range(2):
                    for sh in range(2):
                        dst = outs[pair][:, sh * 1024:(sh + 1) * 1024]
                        if bias_sb is not None:
                            nc.vector.tensor_scalar(
                                dst, acc[pair][sh],
                                bias_sb[:, pair:pair + 1], None, OP.add)
                        else:
                            nc.scalar.activation(dst, acc[pair][sh], AF.Copy)

            project(qt_d, wqt_sb, qT, bq_sb, f32r)
            project(kt_d, wkt_sb, kT, bk_sb, f32r)
            project(vt_d, wvt_sb, vTp, None, bf16)
            # vT [128(2hd), S] -> v natural [128(s in chunk), chunk, hd] via xbar
            for pair in range(2):
                nc.sync.dma_start(v_sb[pair][:], vTp[pair][:], transpose=True)

            # ---------- attention ----------
            ctxT = [sb.tile([128, S], f32r, name=f"ctxT{p}", tag="ctxT", bufs=2)
                    for p in range(2)]
            eT = [sb.tile([128, NC, 2, 128], bf16, name=f"eT{h}", tag="eT",
                          bufs=4)
                  for h in range(HPC)]

            for t in range(NT):
                mt = sb.tile([128, S], bf16, name="mt", tag="mtile", bufs=2)
                nc.sync.dma_start(mt[:], mstr_d[t * 128:(t + 1) * 128, :])
                for h in range(HPC):
                    pair, sub = h // 2, h % 2
                    po = sub * 64
                    # scores: [128, S] in two [128,1024] psum tiles, K=64
                    ps = [pp.tile([128, 1024], f32, name=f"ps{s}", tag="ps",
                                  bufs=4)
                          for s in range(2)]
                    lhsT = qT[pair][po:po + 64, t * 128:(t + 1) * 128]
                    for sh in range(2):
                        for n in range(2):
                            nc.tensor.matmul(
                                ps[sh][:, n * 512:(n + 1) * 512],
                                lhsT,
                                kT[pair][
                                    po:po + 64,
                                    sh * 1024 + n * 512:sh * 1024 + (n + 1) * 512],
                                start=True, stop=True)
                    # mask add (PSUM f32 + SBUF bf16 -> SBUF f32)
                    smask = sb.tile([128, S], f32, name="smask", tag="f32t",
                                    bufs=5)
                    for sh in range(2):
                        nc.vector.tensor_tensor(
                            smask[:, sh * 1024:(sh + 1) * 1024], ps[sh],
                            mt[:, sh * 1024:(sh + 1) * 1024], OP.add)
                    # exp + row sums
                    e = sb.tile([128, S], f32, name="e", tag="f32t", bufs=5)
                    z = sb.tile([128, 4], f32, name="z", tag="z", bufs=3)
                    nc.scalar.activation(e[:], smask[:], AF.Exp,
                                         accum_out=z[:, 0:1])
                    # inv = 2048 / Z
                    nc.vector.tensor_scalar(z[:, 1:2], z[:, 0:1],
                                            1.0 / float(S), None, OP.mult)
                    nc.vector.reciprocal(z[:, 2:3], z[:, 1:2])
                    # attn f32 -> HBM
                    attn32 = sb.tile([128, S], f32, name="attn32", tag="f32t",
                                     bufs=5)
                    nc.vector.tensor_scalar(attn32[:], e[:], z[:, 2:3], None,
                                            OP.mult)
                    nc.sync.dma_start(attnw_d[h, t * 128:(t + 1) * 128, :],
                                      attn32[:])
                    # attn bf16 (for transpose): split halves DVE/ACT
                    attn16 = sb.tile([128, S], bf16, name="attn16", tag="bf4k",
                                     bufs=6)
                    nc.scalar.activation(attn16[:, 0:1024], e[:, 0:1024],
                                         AF.Copy, scale=z[:, 2:3])
                    nc.vector.tensor_scalar(attn16[:, 1024:2048],
                                            e[:, 1024:2048], z[:, 2:3], None,
                                            OP.mult)
                    # xbar block-transpose into eT[h][:, c, t%2, :]
                    nc.sync.dma_start(eT[h][:, :, t % 2, :], attn16[:],
                                      transpose=True)
                    # ctx for the completed 2-tile group
                    if t % 2 == 1:
                        g = t // 2
                        cps = pp.tile([64, 256], f32, name="cps", tag="ps",
                                      bufs=4)
                        for c in range(NC):
                            nc.tensor.matmul(
                                cps[:],
                                v_sb[pair][:, c, po:po + 64],
                                eT[h][:, c, :, :],
                                start=(c == 0), stop=(c == NC - 1))
                        nc.vector.tensor_copy(
                            ctxT[pair][po:po + 64, g * 256:(g + 1) * 256],
                            cps[:])

            # ---------- output projection (transposed) ----------
            for od in range(8):
                for qh in range(2):
                    ops = pp.tile([128, 1024], f32, name="ops", tag="ps",
                                  bufs=4)
                    for hc in range(2):
                        for n in range(2):
                            nc.tensor.matmul(
                                ops[:, n * 512:(n + 1) * 512],
                                wot_sb[:, hc,
                                                       od * 128:(od + 1) * 128],
                                ctxT[hc][
                                    :, qh * 1024 + n * 512:
                                    qh * 1024 + (n + 1) * 512],
                                start=(hc == 0), stop=(hc == 1))
                    osb = sb.tile([128, 1024], f32, name="osb", tag="osb",
                                  bufs=2)
                    nc.scalar.activation(osb[:], ops[:], AF.Copy)
                    nc.sync.dma_start(
                        outpt_d[od * 128:(od + 1) * 128,
                                qh * 1024:(qh + 1) * 1024], osb[:])

    nc.compile()
    return nc


def _get_module():
    if "nc" not in _CACHE:
        _CACHE["nc"] = _build_module()
    return _CACHE["nc"]


def _shard_inputs(Q, K, V, attn_mask, Wq, bq, Wk, bk, Wv, bv, Wo, bo):
    """Host-side sharding: one input dict per core."""
    bf16 = ml_dtypes.bfloat16
    scale = np.float32(1.0 / np.sqrt(DK))
    in_maps = []
    for c in range(NCORES):
        b = c // GROUPS_PER_B
        h0 = (c % GROUPS_PER_B) * HPC
        rows = slice(h0 * DK, (h0 + HPC) * DK)
        m = {
            "qt": np.ascontiguousarray(Q[b].T).astype(np.float32),
            "kt": np.ascontiguousarray(K[b].T).astype(np.float32),
            "vt": np.ascontiguousarray(V[b].T).astype(bf16),
            "mstr": np.where(attn_mask[b], MASK_NEG, np.float32(0)).astype(bf16),
            "wqt": np.ascontiguousarray((Wq[rows] * scale).T).astype(np.float32),
            "wkt": np.ascontiguousarray(Wk[rows].T).astype(np.float32),
            "wvt": np.ascontiguousarray(Wv[rows].T).astype(bf16),
            "wot": np.ascontiguousarray(Wo[:, rows].T).astype(np.float32),
            "bqs": (bq[rows] * scale).astype(np.float32).reshape(-1, 1),
            "bks": bk[rows].astype(np.float32).reshape(-1, 1),
        }
        in_maps.append(m)
    return in_maps


def kernel(Q, K, V, attn_mask, Wq, bq, Wk, bk, Wv, bv, Wo, bo):
    from concourse.bass_utils import run_bass_kernel_spmd

    Q = np.asarray(Q, dtype=np.float32)
    K = np.asarray(K, dtype=np.float32)
    V = np.asarray(V, dtype=np.float32)
    attn_mask = np.asarray(attn_mask)
    Wq = np.asarray(Wq, dtype=np.float32)
    Wk = np.asarray(Wk, dtype=np.float32)
    Wv = np.asarray(Wv, dtype=np.float32)
    Wo = np.asarray(Wo, dtype=np.float32)
    bq = np.asarray(bq, dtype=np.float32)
    bk = np.asarray(bk, dtype=np.float32)
    bv = np.asarray(bv, dtype=np.float32)
    bo = np.asarray(bo, dtype=np.float32)

    nc = _get_module()
    in_maps = _shard_inputs(Q, K, V, attn_mask, Wq, bq, Wk, bk, Wv, bv, Wo, bo)
    res = run_bass_kernel_spmd(nc, in_maps, core_ids=list(range(NCORES)))
    results = res.results
    if res.exec_time_ns is not None:
        _CACHE["exec_time_ns"] = res.exec_time_ns

    attn_weights = np.empty((B, H, S, S), dtype=np.float32)
    output = np.empty((B, S, D), dtype=np.float32)
    # constant term: v-bias through Wo (+bo); attn rows sum to exactly S
    const = (np.float32(S) * (bv @ Wo.T) + bo).astype(np.float32)
    for b in range(B):
        acc = None
        for g in range(GROUPS_PER_B):
            c = b * GROUPS_PER_B + g
            h0 = g * HPC
            attn_weights[b, h0:h0 + HPC] = results[c]["attnw"]
            acc = results[c]["outpt"] if acc is None else acc + results[c]["outpt"]
        output[b] = acc.T + const
    return output, attn_weights
